# revision 32
# baseline (speedup 1.0000x reference)
"""MoE SwiGLU FFN (grouped GEMM) Trainium2 kernel.

Problem: E=32 experts, T=65536 tokens pre-sorted by expert (uniform 2048
tokens/expert), D=512, H=1024.
    h1 = ragged_dot(x, w1) + b1[seg]; h3 = ragged_dot(x, w3) + b3[seg]
    out = ragged_dot(silu(h1)*h3, w2) + b2[seg]

Sharding: expert parallelism across 8 cores. Tokens are pre-sorted and
uniformly dispatched, so expert-parallel == token-parallel: core c owns
experts [4c, 4c+4) and token rows [8192c, 8192(c+1)). No collectives.

Per-core kernel (all matmuls bf16 with fp32 PSUM accumulation; bf16
streams 1 col/cycle with LDWEIGHTS hidden by FWL - 216 vs 232 ns per
[128x128]@[128x512] matmul vs fp32r; end-to-end rel l2 = 4.4e-3, well
inside the 2e-2 gate, and bf16 halves HBM traffic since x/w/out are
host-cast):
  - x slab is passed host-transposed+cast as xt [D, TPC] bf16 so every
    DMA is a contiguous-row load (contraction over D needs D on
    partitions); w1/w3/w2 are host-cast to bf16, b1/b3 host-transposed
    to [P, MH] (contiguous bias loads), b2 host-broadcast to [P, D].
  - DMA triggers cost ~650 ns of issuing-engine time each, so DMAs are
    aggregated: one load per weight tensor per expert (expert 0 splits
    w1/w3 into H-halves so the first m-tiles' matmuls start ~1.5 us
    earlier), one x load per 512-token chunk, one out store per chunk.
  - queue split: weights ride SWDGE (gpsimd), x + biases + out stores
    ride the SP HWDGE (sync) queue, and the Act queue carries ONLY silu
    so no DMA-trigger head-of-line blocking delays the SwiGLU eviction
    chain (which PSUM-bank recycling depends on).
  - GEMM1/3 produce H^T tiles [H-part, token-free]; ACT applies
    silu(psum1+b1) (bias is per-partition in this layout), DVE fuses
    (psum3+b3)*silu -> bf16 in one scalar_tensor_tensor op; GEMM2 then
    contracts H on partitions with no further transposes, and DVE evicts
    psum2 + b2 -> bf16 out slices, stored once per chunk.
  - chunk-level software pipeline: GEMM2 of chunk i is emitted after
    GEMM1/3 of chunk i+1 so PE never waits on the SwiGLU tail.
"""

import ml_dtypes
import numpy as np

import concourse.bass as bass
import concourse.mybir as mybir
from concourse.bass_utils import run_bass_kernel_spmd
from concourse.tile import TileContext

BF16_NP = ml_dtypes.bfloat16

E, T, D, H = 32, 65536, 512, 1024
NCORES = 8
EPC = E // NCORES        # experts per core
TPC = T // NCORES        # tokens per core
TPE = T // E             # tokens per expert
NT = 512                 # token chunk (one PSUM bank in fp32)
P = 128
H2 = H // 2

FP32 = mybir.dt.float32
BF16 = mybir.dt.bfloat16
DT_MM = BF16
AF = mybir.ActivationFunctionType
ALU = mybir.AluOpType

KD = D // P              # 4 k-tiles for GEMM1/3
KH = H // P              # 8 k-tiles for GEMM2
MH = H // P              # 8 h m-tiles per chunk
NCHUNK = TPE // NT       # 4 chunks per expert
MT = NT // P             # 4 token sub-tiles per chunk


def _split_sync_waits(nc, max_waits=1):
    """The external neuronxcc walrus only accepts one sync-wait command per
    instruction; hoist excess waits onto preceding NoOps on the same engine."""
    n = 0
    for fn in nc.m.functions:
        for bb in fn.blocks:
            insts = bb.instructions
            i = 0
            while i < len(insts):
                inst = insts[i]
                si = inst.sync_info
                if si is not None and len(si.on_wait) > max_waits:
                    waits = list(si.on_wait)
                    while len(waits) > max_waits:
                        chunk, waits = waits[:max_waits], waits[max_waits:]
                        nop = mybir.InstNoOp(name=f"wait-split-{n}", ins=[], outs=[])
                        n += 1
                        nop.engine = inst.engine
                        nop.sync_info = mybir.SyncInfo(on_wait=chunk, on_update=[])
                        insts.insert(i, nop)
                        i += 1
                    inst.sync_info = mybir.SyncInfo(on_wait=waits, on_update=si.on_update)
                i += 1
    return n


def build_nc():
    nc = bass.Bass()

    xt = nc.declare_dram_parameter("xt", [D, TPC], BF16, isOutput=False)
    w1 = nc.declare_dram_parameter("w1", [EPC, D, H], BF16, isOutput=False)
    b1 = nc.declare_dram_parameter("b1", [EPC, P, MH], FP32, isOutput=False)
    w3 = nc.declare_dram_parameter("w3", [EPC, D, H], BF16, isOutput=False)
    b3 = nc.declare_dram_parameter("b3", [EPC, P, MH], FP32, isOutput=False)
    w2 = nc.declare_dram_parameter("w2", [EPC, H, D], BF16, isOutput=False)
    b2 = nc.declare_dram_parameter("b2", [EPC, P, D], BF16, isOutput=False)
    out = nc.declare_dram_parameter("out", [TPC, D], BF16, isOutput=True)

    with TileContext(nc) as tc:
        with (
            tc.tile_pool(name="w1p", bufs=2) as w1pool,
            tc.tile_pool(name="w3p", bufs=2) as w3pool,
            tc.tile_pool(name="w2p", bufs=2) as w2pool,
            tc.tile_pool(name="bias", bufs=2) as bias_pool,
            tc.tile_pool(name="xp", bufs=2) as xpool,
            tc.tile_pool(name="hp", bufs=2 * MH) as hpool,
            tc.tile_pool(name="t1p", bufs=MH) as t1pool,
            tc.tile_pool(name="op", bufs=2) as opool,
            tc.tile_pool(name="ps13", bufs=6, space="PSUM") as ps13,
            tc.tile_pool(name="ps2", bufs=2, space="PSUM") as ps2,
            tc.tile_pool(name="w0p", bufs=1) as w0pool,
            tc.tile_pool(name="warm", bufs=1) as warm_pool,
        ):
            # --- PE clock warmup -------------------------------------------
            # The PE runs at 1.2 GHz until it has been busy for a ~3.4 us HAM
            # window. The first ~3 us after the framework preamble are an
            # unavoidable DMA fill (weights/x in flight), so fill them with
            # tiny K=1 N=64 matmuls on memset const tiles: by the time real
            # matmuls start (~10.5 us) the clock is already at 2.4 GHz.
            wlhs = warm_pool.tile([P, P], DT_MM, tag="wl", name="warm_lhs")
            nc.vector.memset(wlhs[:], 1.0)
            wrhs = warm_pool.tile([P, 64], DT_MM, tag="wr", name="warm_rhs")
            nc.vector.memset(wrhs[:], 1.0)
            wps = ps2.tile([P, D], FP32, tag="p2", name="warm_ps")
            for wi in range(64):
                nc.tensor.matmul(wps[:, 0:64], lhsT=wlhs[:], rhs=wrhs[:],
                                 start=True, stop=True)

            def load_w13(e):
                """GEMM1/3 weights: one DMA per tensor ([P, KD, H] viewed).
                Expert 0 instead loads per-k [P, H] tiles (8 triggers) so the
                PE can chase the DMA wavefront: m0's k0 matmul starts after
                256 KB + 128 KB of x instead of after the full 1 MB."""
                if e == 0:
                    # expert 0 rides the Act HWDGE queue: ~0.6us first-byte
                    # vs SWDGE's ~1.6us, and the Act engine has no silu work
                    # until these matmuls produce psum anyway.
                    k1t, k3t = [], []
                    for k in range(KD):
                        t = w0pool.tile([P, H], DT_MM, tag=f"w1k{k}",
                                        name=f"w1k_{k}")
                        nc.scalar.dma_start(out=t[:], in_=w1[e, k * P:(k + 1) * P, :])
                        k1t.append(t)
                    for k in range(KD):
                        t = w0pool.tile([P, H], DT_MM, tag=f"w3k{k}",
                                        name=f"w3k_{k}")
                        nc.scalar.dma_start(out=t[:], in_=w3[e, k * P:(k + 1) * P, :])
                        k3t.append(t)

                    def slice1(k, m, _t=k1t):
                        return _t[k][:, m * P:(m + 1) * P]

                    def slice3(k, m, _t=k3t):
                        return _t[k][:, m * P:(m + 1) * P]
                else:
                    t1_ = w1pool.tile([P, KD, H], DT_MM, tag="w1f", name=f"w1f_{e}")
                    nc.gpsimd.dma_start(
                        out=t1_[:], in_=w1[e].rearrange("(k p) h -> p k h", p=P))
                    t3_ = w3pool.tile([P, KD, H], DT_MM, tag="w3f", name=f"w3f_{e}")
                    nc.gpsimd.dma_start(
                        out=t3_[:], in_=w3[e].rearrange("(k p) h -> p k h", p=P))

                    def slice1(k, m, _t=t1_):
                        return _t[:, k, m * P:(m + 1) * P]

                    def slice3(k, m, _t=t3_):
                        return _t[:, k, m * P:(m + 1) * P]

                b1s = bias_pool.tile([P, MH], FP32, tag="b1", name=f"b1_{e}")
                nc.sync.dma_start(out=b1s[:], in_=b1[e])
                b3s = bias_pool.tile([P, MH], FP32, tag="b3", name=f"b3_{e}")
                nc.sync.dma_start(out=b3s[:], in_=b3[e])
                return dict(w1=slice1, w3=slice3, b1=b1s, b3=b3s)

            def load_w2b2(e, wts):
                t = w2pool.tile([P, KH, D], DT_MM, tag="w2f", name=f"w2f_{e}")
                nc.gpsimd.dma_start(
                    out=t[:], in_=w2[e].rearrange("(k p) d -> p k d", p=P))
                b2b = bias_pool.tile([P, D], BF16, tag="b2b", name=f"b2b_{e}")
                nc.sync.dma_start(out=b2b[:], in_=b2[e])
                wts["w2"] = t
                wts["b2"] = b2b

            def emit_xload(e, c):
                t0 = e * TPE + c * NT
                xb = xpool.tile([P, KD, NT], DT_MM, tag="x", name=f"x_{e}_{c}")
                nc.sync.dma_start(
                    out=xb[:],
                    in_=xt.rearrange("(k p) t -> p k t", p=P)[:, :, t0:t0 + NT])
                return xb

            def emit_gemm13(e, c, wts, xb, phase_split=False):
                """GEMM1+GEMM3+SwiGLU for chunk c of expert e -> 8 bf16 H^T tiles.

                phase_split (first chunk only): emit ALL GEMM1 m-tiles (+silu)
                before any GEMM3 so the in-order PE consumes w1 then w3 -- the
                exact DMA arrival order -- instead of stalling on w3 after
                every 4 matmuls. Evictions still fire per m-tile, so PSUM
                banks recycle; t1 tiles (MH bufs) carry silu across phases."""
                def g1(m):
                    p1 = ps13.tile([P, NT], FP32, tag="p13", name=f"p1_{e}_{c}_{m}")
                    for k in range(KD):
                        nc.tensor.matmul(
                            p1[:], lhsT=wts["w1"](k, m), rhs=xb[:, k, :],
                            start=(k == 0), stop=(k == KD - 1))
                    t1 = t1pool.tile([P, NT], FP32, tag="t1", name=f"t1_{e}_{c}_{m}")
                    nc.scalar.activation(t1[:], p1[:], AF.Silu,
                                         bias=wts["b1"][:, m:m + 1], scale=1.0)
                    return t1

                def g3(m, t1):
                    p3 = ps13.tile([P, NT], FP32, tag="p13", name=f"p3_{e}_{c}_{m}")
                    for k in range(KD):
                        nc.tensor.matmul(
                            p3[:], lhsT=wts["w3"](k, m), rhs=xb[:, k, :],
                            start=(k == 0), stop=(k == KD - 1))
                    hbf = hpool.tile([P, NT], DT_MM, tag="h", name=f"h_{e}_{c}_{m}")
                    nc.vector.scalar_tensor_tensor(
                        out=hbf[:], in0=p3[:], scalar=wts["b3"][:, m:m + 1], in1=t1[:],
                        op0=ALU.add, op1=ALU.mult)
                    return hbf

                if phase_split:
                    t1s = [g1(m) for m in range(MH)]
                    return [g3(m, t1s[m]) for m in range(MH)]
                htiles = []
                for m in range(MH):
                    t1 = g1(m)
                    htiles.append(g3(m, t1))
                return htiles

            def emit_gemm2(e, c, wts, htiles, split_store=False):
                t0 = e * TPE + c * NT
                ot = opool.tile([P, MT, D], BF16, tag="o", name=f"o_{e}_{c}")
                for mt in range(MT):
                    p2 = ps2.tile([P, D], FP32, tag="p2", name=f"p2_{e}_{c}_{mt}")
                    for k in range(KH):
                        nc.tensor.matmul(
                            p2[:], lhsT=htiles[k][:, mt * P:(mt + 1) * P],
                            rhs=wts["w2"][:, k, :],
                            start=(k == 0), stop=(k == KH - 1))
                    nc.vector.tensor_add(ot[:, mt, :], p2[:], wts["b2"][:])
                    if split_store and mt == 1:
                        nc.sync.dma_start(
                            out=out[t0:t0 + 2 * P, :].rearrange(
                                "(mt p) d -> p mt d", p=P),
                            in_=ot[:, 0:2, :])
                if split_store:
                    nc.sync.dma_start(
                        out=out[t0 + 2 * P:t0 + NT, :].rearrange(
                            "(mt p) d -> p mt d", p=P),
                        in_=ot[:, 2:4, :])
                else:
                    nc.sync.dma_start(
                        out=out[t0:t0 + NT, :].rearrange("(mt p) d -> p mt d", p=P),
                        in_=ot[:])

            # chunk-level pipeline across the whole (expert, chunk) sequence.
            # x of (0,0) is emitted before anything else so its trigger leads
            # the sync queue and the first matmul isn't gated on bias loads.
            xb_next = emit_xload(0, 0)
            pending = None  # (e, c, wts, htiles)
            for e in range(EPC):
                wts_e = load_w13(e)
                for c in range(NCHUNK):
                    xb = xb_next
                    h = emit_gemm13(e, c, wts_e, xb,
                                    phase_split=(e == 0 and c == 0))
                    if e == 0 and c == 0:
                        # Gate the SWDGE queue on the first SwiGLU output:
                        # everything queued on gpsimd after this (w2 of e0,
                        # all of e1..e3's weights — ~10 MB) stays off HBM
                        # until ~15 us, keeping the startup-critical w1/w3/x
                        # streams at full bandwidth.
                        gate = warm_pool.tile([1, 1], DT_MM, tag="gate",
                                              name="swdge_gate")
                        nc.gpsimd.tensor_copy(gate[:], h[0][0:1, 0:1])
                    if not (e == EPC - 1 and c == NCHUNK - 1):
                        xb_next = emit_xload(*divmod(e * NCHUNK + c + 1, NCHUNK))
                    if c == 0:
                        load_w2b2(e, wts_e)
                    if pending is not None:
                        emit_gemm2(*pending)
                    pending = (e, c, wts_e, h)
            emit_gemm2(*pending, split_store=True)

    _split_sync_waits(nc)
    return nc


_NC_CACHE = {}


def _get_nc():
    if "nc" not in _NC_CACHE:
        _NC_CACHE["nc"] = build_nc()
    return _NC_CACHE["nc"]


def _kernel_np_fallback(x, w1, b1, w3, b3, w2, b2, group_sizes):
    """Numpy reference path for non-uniform group sizes (not expected)."""
    bounds = np.cumsum(group_sizes)
    seg = np.searchsorted(bounds, np.arange(x.shape[0]), side="right")
    out = np.empty((x.shape[0], w2.shape[2]), np.float32)
    start = 0
    for e in range(len(group_sizes)):
        stop = start + int(group_sizes[e])
        xs = x[start:stop]
        h1 = xs @ w1[e] + b1[e]
        h3 = xs @ w3[e] + b3[e]
        h = (h1 / (1.0 + np.exp(-h1))) * h3
        out[start:stop] = h @ w2[e] + b2[e]
        start = stop
    return out


def prep_in_maps(inputs):
    """Shard + host-side layout: xt transposed bf16 x slab per core; w* bf16;
    b1/b3 transposed to [P, MH]; b2 broadcast to [P, D] bf16."""
    x = np.asarray(inputs["x"], np.float32)
    w1 = np.asarray(inputs["w1"]).astype(BF16_NP)
    w3 = np.asarray(inputs["w3"]).astype(BF16_NP)
    w2 = np.asarray(inputs["w2"]).astype(BF16_NP)
    b1t = np.ascontiguousarray(
        np.asarray(inputs["b1"], np.float32).reshape(E, MH, P).transpose(0, 2, 1))
    b3t = np.ascontiguousarray(
        np.asarray(inputs["b3"], np.float32).reshape(E, MH, P).transpose(0, 2, 1))
    b2full = np.ascontiguousarray(np.broadcast_to(
        np.asarray(inputs["b2"]).astype(BF16_NP)[:, None, :], (E, P, D)))
    in_maps = []
    for c in range(NCORES):
        es = slice(c * EPC, (c + 1) * EPC)
        in_maps.append(dict(
            xt=x[c * TPC:(c + 1) * TPC].T.astype(BF16_NP, order="C"),
            w1=np.ascontiguousarray(w1[es]),
            b1=b1t[es],
            w3=np.ascontiguousarray(w3[es]),
            b3=b3t[es],
            w2=np.ascontiguousarray(w2[es]),
            b2=b2full[es],
        ))
    return in_maps


def kernel(x, w1, b1, w3, b3, w2, b2, group_sizes):
    gs = np.asarray(group_sizes)
    if not (gs.shape == (E,) and np.all(gs == T // E) and x.shape == (T, D)):
        return _kernel_np_fallback(np.asarray(x, np.float32), w1, b1, w3, b3,
                                   w2, b2, gs).astype(np.float32)

    in_maps = prep_in_maps(dict(x=x, w1=w1, b1=b1, w3=w3, b3=b3, w2=w2, b2=b2))
    nc = _get_nc()
    res = run_bass_kernel_spmd(nc, in_maps, list(range(NCORES)))
    return np.concatenate(
        [res.results[c]["out"] for c in range(NCORES)], axis=0
    ).astype(np.float32)
